# revision 41
# baseline (speedup 1.0000x reference)
import os
import sys

for _p in ("/opt/trn_rl_repo", os.path.expanduser("~/.axon_site/_ro/trn_rl_repo")):
    if os.path.isdir(_p) and _p not in sys.path:
        sys.path.insert(0, _p)

import numpy as np
import ml_dtypes

import concourse.bass as bass
from concourse import bacc
import concourse.tile as tile
import concourse.mybir as mybir
from concourse.bass_utils import run_bass_kernel_spmd

# Problem shape (hardcoded per contract)
B, T, D, H, DK = 4, 2048, 1024, 16, 64
NCORES = 8

# Sharding: core = (batch b, head-group hg). Each core handles 8 heads of one
# batch over the full sequence, row-shards W_o, and the host sums the two
# partial outputs per batch (the "all-reduce" of the tensor-parallel scheme).
HC = H // 2       # 8 heads per core
DC = HC * DK      # 512 hidden dims per core

P = 128
NDT = D // P      # 8 din tiles
NHT = DC // P     # 4 dout tiles (= head pairs) for this core's heads
NKT = T // P      # 16 key-token tiles
NPAIR = HC // 2   # 4 head pairs (pair p <-> dout tile p)
QCH = 512         # free-dim chunk per q-chunk
NQC = T // QCH    # 4 q-chunks
NG = NKT // 2     # 8 kt-groups (2 kt each) per (chunk, pair) block

bf16 = mybir.dt.bfloat16
fp8 = mybir.dt.float8e4
f32 = mybir.dt.float32
FT = mybir.ActivationFunctionType
ADD = mybir.AluOpType.add
MUL = mybir.AluOpType.mult
DR = mybir.MatmulPerfMode.DoubleRow

_CACHE = {}

# Software-pipeline tuning: PE cycle budget granted per exp-period. The Act
# engine (exp stream) paces the kernel at ~2076 ns / period (~4980 PE cycles
# at 2.4 GHz); scores, AV chains and filler items all charge against it.
SLACK_CYC = 4400
BUDGET_CAP = 1500
PROLOGUE_BUDGET = 10000


def build_kernel():
    nc = bacc.Bacc("TRN2", target_bir_lowering=False, debug=False, num_devices=1)

    # Per-core inputs, pre-tiled on the host so every DMA is one contiguous
    # transfer (HWDGE dispatch is ~625ns per DMA, so fewer/bigger is better):
    # Wq/Wk: [p, dt, a, m] = W[a*128+p, dt*128+m]; Wv: [p, a, m] = W[a*128+p, m]
    # Wo: [p, ch, a, m] = Wo_shard[a*128+p, ch*512+m]
    xT = nc.dram_tensor("xT", [D, T], bf16, kind="ExternalInput")  # x[b].T
    Wq = nc.dram_tensor("Wq", [P, NHT, NDT, P], bf16, kind="ExternalInput")
    Wk = nc.dram_tensor("Wk", [P, NHT, NDT, P], bf16, kind="ExternalInput")
    Wv = nc.dram_tensor("Wv", [P, NDT, DC], bf16, kind="ExternalInput")
    Wo = nc.dram_tensor("Wo", [P, 2, NHT, QCH], bf16, kind="ExternalInput")
    # bq pre-striped on host to [128, NHT] (col t = bias[t*128:(t+1)*128]).
    # The K bias is dropped entirely: softmax over keys is invariant to the
    # (q+bq)@bk term, so scores use (q+bq)@k with k = x@Wk (no bias).
    bqp = nc.dram_tensor("bqp", [P, NHT], f32, kind="ExternalInput")
    bv = nc.dram_tensor("bv", [1, DC], f32, kind="ExternalInput")
    ident = nc.dram_tensor("ident", [P, P], bf16, kind="ExternalInput")
    bo = nc.dram_tensor("bo", [1, D], bf16, kind="ExternalInput")  # pre-halved
    out = nc.dram_tensor("out", [T, D], mybir.dt.float16, kind="ExternalOutput")

    xTv = xT[:].rearrange("(a p) t -> p a t", p=P)  # [128, NDT, T] view

    with tile.TileContext(nc) as tc:
        with (
            tc.tile_pool(name="big", bufs=1) as big,
            tc.tile_pool(name="ptp", bufs=12) as ptp,
            tc.tile_pool(name="opk", bufs=8) as opkp,
            tc.tile_pool(name="rec", bufs=2) as recp,
            tc.tile_pool(name="part", bufs=4) as partp,
            tc.tile_pool(name="res", bufs=4) as resp,
            tc.tile_pool(name="sg", bufs=1, space="PSUM") as sgp,
            tc.tile_pool(name="acc", bufs=1, space="PSUM") as accp,
            tc.tile_pool(name="ops", bufs=2, space="PSUM") as opsp,
        ):
            # ---------------- static SBUF tiles + input DMA ----------------
            wk_w = big.tile([P, NHT, NDT, P], bf16, name="wk_w")
            wq_w = big.tile([P, NHT, NDT, P], bf16, name="wq_w")
            xt_sb = big.tile([P, NDT, T], bf16, name="xt_sb")
            bq_sb = big.tile([P, NHT], f32, name="bq_sb")
            bv_rep = big.tile([P, DC], f32, name="bv_rep")
            bo_rep = big.tile([P, D], bf16, name="bo_rep")
            wv_ch = big.tile([P, NDT, DC], bf16, name="wv_ch")
            wo_ch = big.tile([P, 2, NHT, QCH], bf16, name="wo_ch")

            # fp8 Q/K for DoubleRow scores. Layout [p, i, t]: partition p<64 =
            # head-A dims, p>=64 = head-B dims; i is the DoubleRow k-tile dim
            # (tile 1 kept zero so contraction-64 runs at 0.5 cyc/row).
            id_sb = big.tile([P, P], bf16, name="id_sb")
            kt_f8 = [big.tile([P, 2, T], fp8, name=f"ktf{p}") for p in range(NPAIR)]
            qt_f8 = [big.tile([P, 2, T], fp8, name=f"qtf{p}") for p in range(NPAIR)]
            # V with bias, ones column at [:, h, 64] for softmax denominators
            vp_sb = [big.tile([P, HC, DK + 1], bf16, name=f"vp{t}") for t in range(NKT)]
            # attention outputs, transposed: [din of pair, tt, token]
            ob_sb = [big.tile([P, NKT, P], bf16, name=f"ob{p}") for p in range(NPAIR)]

            # input loads, priority order, all on SP/HWDGE. The first
            # scores' critical path is x[chunk0] + Wk[pair0] -> Wq[pair0];
            # chunk0 is split so the K matmuls can start on the first half.
            nc.sync.dma_start(xt_sb[:, 0 : NDT // 2, 0:QCH], xTv[:, 0 : NDT // 2, 0:QCH])
            nc.sync.dma_start(wk_w[:, 0], Wk[:, 0])
            nc.sync.dma_start(xt_sb[:, NDT // 2 :, 0:QCH], xTv[:, NDT // 2 :, 0:QCH])
            nc.sync.dma_start(wq_w[:, 0], Wq[:, 0])
            nc.sync.dma_start(bq_sb[:], bqp[:])
            nc.sync.dma_start(xt_sb[:, :, QCH : 2 * QCH], xTv[:, :, QCH : 2 * QCH])
            nc.sync.dma_start(bv_rep[:], bv[:].to_broadcast((P, DC)))
            nc.sync.dma_start(wv_ch[:], Wv[:])
            nc.sync.dma_start(xt_sb[:, :, 2 * QCH : 3 * QCH], xTv[:, :, 2 * QCH : 3 * QCH])
            nc.sync.dma_start(wk_w[:, 1:NHT], Wk[:, 1:NHT])
            nc.sync.dma_start(xt_sb[:, :, 3 * QCH :], xTv[:, :, 3 * QCH :])
            nc.sync.dma_start(wq_w[:, 1:NHT], Wq[:, 1:NHT])
            nc.sync.dma_start(wo_ch[:], Wo[:])
            nc.sync.dma_start(bo_rep[:], bo[:].to_broadcast((P, D)))
            nc.sync.dma_start(id_sb[:], ident[:])

            # memsets on Pool (otherwise idle): DoubleRow zero-tiles for pair 0
            # first (needed by the first scores), then denominators' ones
            # column, then the remaining pairs.
            nc.gpsimd.memset(kt_f8[0][:, 1, :], 0.0)
            nc.gpsimd.memset(qt_f8[0][:, 1, :], 0.0)
            for t in range(NKT):
                nc.gpsimd.memset(vp_sb[t][:, :, DK : DK + 1], 1.0)
            for p in range(1, NPAIR):
                nc.gpsimd.memset(kt_f8[p][:, 1, :], 0.0)
                nc.gpsimd.memset(qt_f8[p][:, 1, :], 0.0)

            # ---------------- work items (PE filler) ----------------
            kconvs = [0]

            def do_K(p, ch):
                sl = slice(ch * QCH, (ch + 1) * QCH)
                ps = opsp.tile([P, QCH], f32, tag="pps", name="pps")
                for di in range(NDT):
                    nc.tensor.matmul(
                        ps[:], wk_w[:, p, di, :], xt_sb[:, di, sl],
                        start=(di == 0), stop=(di == NDT - 1),
                    )
                if kconvs[0] == 0:
                    # first K: convert in halves so the first scores (keys
                    # 0:256) only wait for the first half
                    h = QCH // 2
                    nc.vector.tensor_copy(kt_f8[p][:, 0, ch * QCH : ch * QCH + h], ps[:, 0:h])
                    nc.vector.tensor_copy(kt_f8[p][:, 0, ch * QCH + h : (ch + 1) * QCH], ps[:, h:])
                else:
                    nc.vector.tensor_copy(kt_f8[p][:, 0, sl], ps[:])
                kconvs[0] += 1

            def do_Q(c, p):
                sl = slice(c * QCH, (c + 1) * QCH)
                ps = opsp.tile([P, QCH], f32, tag="pps", name="pps")
                for di in range(NDT):
                    nc.tensor.matmul(
                        ps[:], wq_w[:, p, di, :], xt_sb[:, di, sl],
                        start=(di == 0), stop=(di == NDT - 1),
                    )
                nc.vector.tensor_tensor(
                    qt_f8[p][:, 0, sl], ps[:],
                    bq_sb[:, p : p + 1].to_broadcast((P, QCH)), ADD,
                )

            def do_V(p, tt):
                # one pair's 128 V columns for token tile tt
                ps = opsp.tile([P, QCH], f32, tag="pps", name="pps")
                csl = slice(p * P, (p + 1) * P)
                for di in range(NDT):
                    nc.tensor.matmul(
                        ps[:, 0:P], xt_sb[:, di, tt * P : (tt + 1) * P],
                        wv_ch[:, di, csl],
                        start=(di == 0), stop=(di == NDT - 1),
                    )
                nc.vector.tensor_tensor(
                    vp_sb[tt][:, 2 * p : 2 * p + 2, 0:DK],
                    ps[:, 0:P].rearrange("q (h d) -> q h d", d=DK),
                    bv_rep[:, csl].rearrange("q (h d) -> q h d", d=DK),
                    ADD,
                )

            out_v = out[:].rearrange("(tt p) d -> p tt d", p=P)
            part_tiles = {}

            def do_OP(ch, tg):
                # 2 token tiles -> one grouped result tile -> one DMA
                res = resp.tile([P, 2, QCH], mybir.dt.float16, tag="ores", name="ores")
                for k in range(2):
                    ttk = 2 * tg + k
                    ps = opsp.tile([P, QCH], f32, tag="pps", name="pps")
                    for p in range(NPAIR):
                        nc.tensor.matmul(
                            ps[:], ob_sb[p][:, ttk, :], wo_ch[:, ch, p, :],
                            start=(p == 0), stop=(p == NPAIR - 1),
                        )
                    nc.vector.tensor_tensor(
                        res[:, k, :], ps[:], bo_rep[:, ch * QCH : (ch + 1) * QCH], ADD
                    )
                nc.sync.dma_start(
                    out_v[:, 2 * tg : 2 * tg + 2, ch * QCH : (ch + 1) * QCH], res[:]
                )

            def do_OPP(ch, tg):
                # partial out-proj (pairs 0..2) + bias, staged to SBUF so the
                # final (pair-3) contribution is all that's left for the tail
                part = partp.tile([P, 2, QCH], bf16, tag="part", name="part")
                for k in range(2):
                    ttk = 2 * tg + k
                    ps = opsp.tile([P, QCH], f32, tag="pps", name="pps")
                    for p in range(NPAIR - 1):
                        nc.tensor.matmul(
                            ps[:], ob_sb[p][:, ttk, :], wo_ch[:, ch, p, :],
                            start=(p == 0), stop=(p == NPAIR - 2),
                        )
                    nc.vector.tensor_tensor(
                        part[:, k, :], ps[:], bo_rep[:, ch * QCH : (ch + 1) * QCH], ADD
                    )
                part_tiles[(ch, tg)] = part

            def do_OPF(ch, tg):
                # tail finals: the staged partial joins the accumulation via
                # an identity matmul, and the PSUM->SBUF copies alternate
                # between Act (idle once the exp stream drains) and DVE
                part = part_tiles.pop((ch, tg))
                res = resp.tile([P, 2, QCH], mybir.dt.float16, tag="ores", name="ores")
                for k in range(2):
                    ttk = 2 * tg + k
                    ps = opsp.tile([P, QCH], f32, tag="pps", name="pps")
                    nc.tensor.matmul(
                        ps[:], ob_sb[NPAIR - 1][:, ttk, :], wo_ch[:, ch, NPAIR - 1, :],
                        start=True, stop=False,
                    )
                    nc.tensor.matmul(
                        ps[:], id_sb[:], part[:, k, :], start=False, stop=True,
                    )
                    if k == 0:
                        nc.scalar.activation(res[:, k, :], ps[:], FT.Copy)
                    else:
                        nc.vector.tensor_copy(res[:, k, :], ps[:])
                nc.sync.dma_start(
                    out_v[:, 2 * tg : 2 * tg + 2, ch * QCH : (ch + 1) * QCH], res[:]
                )

            COSTS = {"K": 4096, "Q": 4096, "V": 1024, "OP": 4096, "OPP": 3072,
                     "OPF": 1024}
            EMIT = {"K": do_K, "Q": do_Q, "V": do_V, "OP": do_OP, "OPP": do_OPP,
                    "OPF": do_OPF}

            state = {"budget": 0}
            emitted = set()
            queue = []  # ordered filler keys

            def emit_item(key):
                if key in emitted:
                    return
                emitted.add(key)
                EMIT[key[0]](*key[1:])
                state["budget"] -= COSTS[key[0]]

            def pump(margin=1 << 30):
                # never start an item that would overdraft the period budget
                # by more than `margin`: a 4096-cycle projection on a nearly
                # spent budget stacks ~2 periods of PE work ahead of the next
                # scores and stalls the exp stream
                while queue and state["budget"] > 0:
                    key = queue[0]
                    if key in emitted:
                        queue.pop(0)
                        continue
                    if COSTS[key[0]] > state["budget"] + margin:
                        break
                    queue.pop(0)
                    emit_item(key)

            # filler queue: V per (pair, tt) so early blocks only need pair 0's
            # V; K chunks get pull-emitted exactly when scores need them.
            for p in range(NPAIR):
                for ch in range(4):
                    queue.append(("K", p, ch))
                for tt in range(NKT):
                    queue.append(("V", p, tt))

            # ---------------- phase 2 machinery ----------------
            blocks = [(c, p) for p in range(NPAIR) for c in range(NQC)]
            pt_tiles = {}     # (bi, g, head) -> pt AP
            av_pending = []   # (bi, qt) in emission order
            norm_cnt = [0] * NQC

            def emit_scores(bi, g):
                c, p = blocks[bi]
                qsl = slice(c * QCH, (c + 1) * QCH)
                for head, base, tag in ((0, 0, "sgA"), (1, 64, "sgB")):
                    sg = sgp.tile([P, 2, QCH], f32, tag=tag, name=tag)
                    for j in range(2):
                        kt = 2 * g + j
                        ksl = slice(kt * P, (kt + 1) * P)
                        nc.tensor.matmul(
                            sg[:, j, :],
                            kt_f8[p][base : base + DK, :, ksl],
                            qt_f8[p][base : base + DK, :, qsl],
                            start=True, stop=True,
                            perf_mode=DR,
                            tile_position=(base, 0),
                        )
                    pt = ptp.tile([P, 2, QCH], bf16, tag=f"pt{head}", name="pt")
                    nc.scalar.activation(pt[:], sg[:], FT.Exp, scale=0.125)
                    pt_tiles[(bi, g, head)] = pt
                state["budget"] -= 1024

            def emit_chain(bi, qt):
                # AV for one query tile: per head, a 16-kt accumulation chain
                # in an exclusive PSUM bank (one open group per 2KB zero
                # region), then normalize + transpose out.
                c, p = blocks[bi]
                if qt == 0:
                    for tt in range(NKT):
                        emit_item(("V", p, tt))
                qsl = slice(qt * P, (qt + 1) * P)
                opk = opkp.tile([P, P], bf16, tag="opk", name="opk")
                for head in (0, 1):
                    acc = accp.tile([P, QCH], f32, tag=f"acc{head}", name="acc")
                    h = 2 * p + head
                    for kt in range(NKT):
                        nc.tensor.matmul(
                            acc[:, 0 : DK + 1],
                            pt_tiles[(bi, kt // 2, head)][:, kt % 2, qsl],
                            vp_sb[kt][:, h, :],
                            start=(kt == 0),
                            stop=(kt == NKT - 1),
                        )
                    rec = recp.tile([P, 1], f32, tag=f"rec{head}", name="rec")
                    nc.vector.reciprocal(rec[:], acc[:, DK : DK + 1])
                    nc.vector.tensor_tensor(
                        opk[:, head * DK : (head + 1) * DK], acc[:, 0:DK],
                        rec[:].to_broadcast((P, DK)), MUL,
                    )
                if bi == len(blocks) - 1:
                    # tail: PE transpose + DVE copy (~0.6us) instead of the
                    # ~2.3us xbar-DMA latency chain
                    tp = opsp.tile([P, P], bf16, tag="pps", name="tp")
                    nc.tensor.matmul(
                        tp[:], opk[:], id_sb[:], start=True, stop=True,
                        is_transpose=True,
                    )
                    nc.vector.tensor_copy(ob_sb[p][:, c * NQC + qt, :], tp[:])
                else:
                    nc.sync.dma_start_transpose(ob_sb[p][:, c * NQC + qt, :], opk[:])
                state["budget"] -= 2080
                if c == NQC - 1 and p == NPAIR - 1 and qt in (1, NQC - 1):
                    # last block: queue each final as soon as its own token
                    # tiles are transposed so finals overlap later chains
                    for ch in range(2):
                        queue.append(("OPF", ch, 2 * c + (0 if qt == 1 else 1)))
                if qt == NQC - 1:
                    for g in range(NG):
                        for head in (0, 1):
                            del pt_tiles[(bi, g, head)]
                    norm_cnt[c] += 1
                    if c == NQC - 1:
                        # last chunk: staged partials once pairs 0..2 done
                        if norm_cnt[c] == NPAIR - 1:
                            for ch in range(2):
                                for tg in (2 * c, 2 * c + 1):
                                    queue.append(("OPP", ch, tg))
                    elif norm_cnt[c] == NPAIR:
                        for ch in range(2):
                            for tg in (2 * c, 2 * c + 1):
                                queue.append(("OP", ch, tg))

            def drain_av():
                cap = 1 if len(av_pending) <= NQC else 2
                n = 0
                while av_pending and n < cap:
                    bi, qt = av_pending[0]
                    if exp_done[0] < (bi + 1) * NG:
                        break
                    av_pending.pop(0)
                    emit_chain(bi, qt)
                    n += 1

            exp_done = [0]

            # ---------------- prologue + main loop ----------------
            # PE p-state warmup: the clock ramps only while the engine is
            # continuously busy, so chew on a zeroed scratch tile during the
            # initial DMA wait to enter the first projections at full speed.
            warm = big.tile([P, QCH], bf16, name="warm")
            nc.vector.memset(warm[:], 0.0)
            for w in range(10):
                wps = opsp.tile([P, QCH], f32, tag="pps", name="pps")
                nc.tensor.matmul(
                    wps[:], warm[:, 0:P], warm[:], start=True, stop=True,
                )

            emit_item(("Q", 0, 0))
            emit_item(("K", 0, 0))
            state["budget"] = PROLOGUE_BUDGET

            for bi, (c, p) in enumerate(blocks):
                emit_item(("Q", c, p))
                for g in range(NG):
                    emit_item(("K", p, g // 2))
                    emit_scores(bi, g)
                    # pull the next K chunk right after this period's scores,
                    # two periods ahead of the scores that will need it
                    emit_item(("K", p, min(NG // 2 - 1, g // 2 + 1)))
                    exp_done[0] += 1
                    # pre-pull the next block's projections (spread across two
                    # early periods) so its first scores are never gated on a
                    # just-emitted K/Q and the burst never delays this block's
                    # own next scores by more than a period
                    if bi + 1 < len(blocks):
                        cn, pn = blocks[bi + 1]
                        if g == 0:
                            emit_item(("K", pn, 0))
                        elif g == 2:
                            emit_item(("Q", cn, pn))
                    drain_av()
                    state["budget"] += SLACK_CYC if bi else SLACK_CYC - 1600
                    pump()
                    if state["budget"] > BUDGET_CAP:
                        state["budget"] = BUDGET_CAP
                for qt in range(NQC):
                    av_pending.append((bi, qt))

            # tail: drain chains with the pump interleaved so out-proj
            # finals start as soon as their token tiles are transposed
            while av_pending:
                bi, qt = av_pending.pop(0)
                state["budget"] = 4000
                emit_chain(bi, qt)
                pump(margin=1 << 30)
            state["budget"] = 1 << 30
            pump(margin=1 << 30)

    nc.compile()
    return nc


def _prep_inputs(x, Wq, bq, Wk, bk, Wv, bv, Wo, bo):
    """Shard + lay out inputs for the 8 cores (batch x head-group)."""
    x = np.asarray(x, dtype=np.float32)
    to_bf = lambda a: np.ascontiguousarray(a).astype(ml_dtypes.bfloat16)
    Wq, Wk, Wv, Wo = (np.asarray(w, np.float32) for w in (Wq, Wk, Wv, Wo))
    bq, bv, bo = (np.asarray(v, np.float32) for v in (bq, bv, bo))
    bo_half = np.ascontiguousarray((bo * 0.5).reshape(1, D)).astype(
        ml_dtypes.bfloat16
    )
    xTb = [to_bf(x[b].T) for b in range(B)]
    in_maps = []
    for core in range(NCORES):
        b, hg = core // 2, core % 2
        csl = slice(hg * DC, (hg + 1) * DC)

        def tile_qk(W):
            # [D, DC] -> [p, dt, a, m]
            return to_bf(
                W[:, csl].reshape(NDT, P, NHT, P).transpose(1, 2, 0, 3)
            )

        in_maps.append(
            {
                "xT": xTb[b],
                "Wq": tile_qk(Wq),
                "Wk": tile_qk(Wk),
                "Wv": to_bf(Wv[:, csl].reshape(NDT, P, DC).transpose(1, 0, 2)),
                "Wo": to_bf(
                    Wo[csl, :].reshape(NHT, P, 2, QCH).transpose(1, 2, 0, 3)
                ),
                "bqp": np.ascontiguousarray(bq[csl].reshape(NHT, P).T),
                "bv": np.ascontiguousarray(bv[csl].reshape(1, DC)),
                "ident": np.eye(P, dtype=ml_dtypes.bfloat16),
                "bo": bo_half,
            }
        )
    return in_maps


def kernel(x, Wq, bq, Wk, bk, Wv, bv, Wo, bo):
    if "nc" not in _CACHE:
        _CACHE["nc"] = build_kernel()
    nc = _CACHE["nc"]
    in_maps = _prep_inputs(x, Wq, bq, Wk, bk, Wv, bv, Wo, bo)
    res = run_bass_kernel_spmd(nc, in_maps, list(range(NCORES)))
    out = np.empty((B, T, D), dtype=np.float32)
    for b in range(B):
        out[b] = res.results[2 * b]["out"].astype(np.float32) + res.results[
            2 * b + 1
        ]["out"].astype(np.float32)
    return out


# revision 44
# speedup vs baseline: 1.0003x; 1.0003x over previous
import os
import sys

for _p in ("/opt/trn_rl_repo", os.path.expanduser("~/.axon_site/_ro/trn_rl_repo")):
    if os.path.isdir(_p) and _p not in sys.path:
        sys.path.insert(0, _p)

import numpy as np
import ml_dtypes

import concourse.bass as bass
from concourse import bacc
import concourse.tile as tile
import concourse.mybir as mybir
from concourse.bass_utils import run_bass_kernel_spmd

# Problem shape (hardcoded per contract)
B, T, D, H, DK = 4, 2048, 1024, 16, 64
NCORES = 8

# Sharding: core = (batch b, head-group hg). Each core handles 8 heads of one
# batch over the full sequence, row-shards W_o, and the host sums the two
# partial outputs per batch (the "all-reduce" of the tensor-parallel scheme).
HC = H // 2       # 8 heads per core
DC = HC * DK      # 512 hidden dims per core

P = 128
NDT = D // P      # 8 din tiles
NHT = DC // P     # 4 dout tiles (= head pairs) for this core's heads
NKT = T // P      # 16 key-token tiles
NPAIR = HC // 2   # 4 head pairs (pair p <-> dout tile p)
QCH = 512         # free-dim chunk per q-chunk
NQC = T // QCH    # 4 q-chunks
NG = NKT // 2     # 8 kt-groups (2 kt each) per (chunk, pair) block

bf16 = mybir.dt.bfloat16
fp8 = mybir.dt.float8e4
f32 = mybir.dt.float32
FT = mybir.ActivationFunctionType
ADD = mybir.AluOpType.add
MUL = mybir.AluOpType.mult
DR = mybir.MatmulPerfMode.DoubleRow

_CACHE = {}

# Software-pipeline tuning: PE cycle budget granted per exp-period. The Act
# engine (exp stream) paces the kernel at ~2076 ns / period (~4980 PE cycles
# at 2.4 GHz); scores, AV chains and filler items all charge against it.
SLACK_CYC = 4400
BUDGET_CAP = 1500
PROLOGUE_BUDGET = 8000


def build_kernel():
    nc = bacc.Bacc("TRN2", target_bir_lowering=False, debug=False, num_devices=1)

    # Per-core inputs, pre-tiled on the host so every DMA is one contiguous
    # transfer (HWDGE dispatch is ~625ns per DMA, so fewer/bigger is better):
    # Wq/Wk: [p, dt, a, m] = W[a*128+p, dt*128+m]; Wv: [p, a, m] = W[a*128+p, m]
    # Wo: [p, ch, a, m] = Wo_shard[a*128+p, ch*512+m]
    xT = nc.dram_tensor("xT", [D, T], bf16, kind="ExternalInput")  # x[b].T
    Wq = nc.dram_tensor("Wq", [P, NHT, NDT, P], bf16, kind="ExternalInput")
    Wk = nc.dram_tensor("Wk", [P, NHT, NDT, P], bf16, kind="ExternalInput")
    Wv = nc.dram_tensor("Wv", [P, NDT, DC], bf16, kind="ExternalInput")
    Wo = nc.dram_tensor("Wo", [P, 2, NHT, QCH], bf16, kind="ExternalInput")
    # bq pre-striped on host to [128, NHT] (col t = bias[t*128:(t+1)*128]).
    # The K bias is dropped entirely: softmax over keys is invariant to the
    # (q+bq)@bk term, so scores use (q+bq)@k with k = x@Wk (no bias).
    bqp = nc.dram_tensor("bqp", [P, NHT], f32, kind="ExternalInput")
    bv = nc.dram_tensor("bv", [1, DC], f32, kind="ExternalInput")
    ident = nc.dram_tensor("ident", [P, P], bf16, kind="ExternalInput")
    bo = nc.dram_tensor("bo", [1, D], bf16, kind="ExternalInput")  # pre-halved
    out = nc.dram_tensor("out", [T, D], mybir.dt.float16, kind="ExternalOutput")

    xTv = xT[:].rearrange("(a p) t -> p a t", p=P)  # [128, NDT, T] view

    with tile.TileContext(nc) as tc:
        with (
            tc.tile_pool(name="big", bufs=1) as big,
            tc.tile_pool(name="ptp", bufs=12) as ptp,
            tc.tile_pool(name="opk", bufs=8) as opkp,
            tc.tile_pool(name="rec", bufs=2) as recp,
            tc.tile_pool(name="part", bufs=4) as partp,
            tc.tile_pool(name="res", bufs=4) as resp,
            tc.tile_pool(name="sg", bufs=1, space="PSUM") as sgp,
            tc.tile_pool(name="acc", bufs=1, space="PSUM") as accp,
            tc.tile_pool(name="ops", bufs=2, space="PSUM") as opsp,
        ):
            # ---------------- static SBUF tiles + input DMA ----------------
            wk_w = big.tile([P, NHT, NDT, P], bf16, name="wk_w")
            wq_w = big.tile([P, NHT, NDT, P], bf16, name="wq_w")
            xt_sb = big.tile([P, NDT, T], bf16, name="xt_sb")
            bq_sb = big.tile([P, NHT], f32, name="bq_sb")
            bv_rep = big.tile([P, DC], f32, name="bv_rep")
            bo_rep = big.tile([P, D], bf16, name="bo_rep")
            wv_ch = big.tile([P, NDT, DC], bf16, name="wv_ch")
            wo_ch = big.tile([P, 2, NHT, QCH], bf16, name="wo_ch")

            # fp8 Q/K for DoubleRow scores. Layout [p, i, t]: partition p<64 =
            # head-A dims, p>=64 = head-B dims; i is the DoubleRow k-tile dim
            # (tile 1 kept zero so contraction-64 runs at 0.5 cyc/row).
            id_sb = big.tile([P, P], bf16, name="id_sb")
            kt_f8 = [big.tile([P, 2, T], fp8, name=f"ktf{p}") for p in range(NPAIR)]
            qt_f8 = [big.tile([P, 2, T], fp8, name=f"qtf{p}") for p in range(NPAIR)]
            # V with bias, ones column at [:, h, 64] for softmax denominators
            vp_sb = [big.tile([P, HC, DK + 1], bf16, name=f"vp{t}") for t in range(NKT)]
            # attention outputs, transposed: [din of pair, tt, token]
            ob_sb = [big.tile([P, NKT, P], bf16, name=f"ob{p}") for p in range(NPAIR)]

            # input loads, priority order, all on SP/HWDGE. The first
            # scores' critical path is x[chunk0] + Wk[pair0] -> Wq[pair0];
            # chunk0 is split so the K matmuls can start on the first half.
            nc.sync.dma_start(xt_sb[:, 0 : NDT // 2, 0:QCH], xTv[:, 0 : NDT // 2, 0:QCH])
            nc.sync.dma_start(wk_w[:, 0], Wk[:, 0])
            nc.sync.dma_start(xt_sb[:, NDT // 2 :, 0:QCH], xTv[:, NDT // 2 :, 0:QCH])
            nc.sync.dma_start(wq_w[:, 0], Wq[:, 0])
            nc.sync.dma_start(bq_sb[:], bqp[:])
            nc.sync.dma_start(xt_sb[:, :, QCH : 2 * QCH], xTv[:, :, QCH : 2 * QCH])
            nc.sync.dma_start(bv_rep[:], bv[:].to_broadcast((P, DC)))
            nc.sync.dma_start(wv_ch[:], Wv[:])
            nc.sync.dma_start(xt_sb[:, :, 2 * QCH : 3 * QCH], xTv[:, :, 2 * QCH : 3 * QCH])
            nc.sync.dma_start(wk_w[:, 1:NHT], Wk[:, 1:NHT])
            nc.sync.dma_start(xt_sb[:, :, 3 * QCH :], xTv[:, :, 3 * QCH :])
            nc.sync.dma_start(wq_w[:, 1:NHT], Wq[:, 1:NHT])
            nc.sync.dma_start(wo_ch[:], Wo[:])
            nc.sync.dma_start(bo_rep[:], bo[:].to_broadcast((P, D)))
            nc.sync.dma_start(id_sb[:], ident[:])

            # memsets on Pool (otherwise idle): DoubleRow zero-tiles for pair 0
            # first (needed by the first scores), then denominators' ones
            # column, then the remaining pairs.
            nc.gpsimd.memset(kt_f8[0][:, 1, :], 0.0)
            nc.gpsimd.memset(qt_f8[0][:, 1, :], 0.0)
            for t in range(NKT):
                nc.gpsimd.memset(vp_sb[t][:, :, DK : DK + 1], 1.0)
            for p in range(1, NPAIR):
                nc.gpsimd.memset(kt_f8[p][:, 1, :], 0.0)
                nc.gpsimd.memset(qt_f8[p][:, 1, :], 0.0)

            # ---------------- work items (PE filler) ----------------
            kconvs = [0]

            def do_K(p, ch):
                sl = slice(ch * QCH, (ch + 1) * QCH)
                ps = opsp.tile([P, QCH], f32, tag="pps", name="pps")
                for di in range(NDT):
                    nc.tensor.matmul(
                        ps[:], wk_w[:, p, di, :], xt_sb[:, di, sl],
                        start=(di == 0), stop=(di == NDT - 1),
                    )
                if kconvs[0] == 0:
                    # first K: convert in halves so the first scores (keys
                    # 0:256) only wait for the first half
                    h = QCH // 2
                    nc.vector.tensor_copy(kt_f8[p][:, 0, ch * QCH : ch * QCH + h], ps[:, 0:h])
                    nc.vector.tensor_copy(kt_f8[p][:, 0, ch * QCH + h : (ch + 1) * QCH], ps[:, h:])
                else:
                    nc.vector.tensor_copy(kt_f8[p][:, 0, sl], ps[:])
                kconvs[0] += 1

            def do_Q(c, p):
                sl = slice(c * QCH, (c + 1) * QCH)
                ps = opsp.tile([P, QCH], f32, tag="pps", name="pps")
                for di in range(NDT):
                    nc.tensor.matmul(
                        ps[:], wq_w[:, p, di, :], xt_sb[:, di, sl],
                        start=(di == 0), stop=(di == NDT - 1),
                    )
                nc.vector.tensor_tensor(
                    qt_f8[p][:, 0, sl], ps[:],
                    bq_sb[:, p : p + 1].to_broadcast((P, QCH)), ADD,
                )

            def do_V(p, tt):
                # one pair's 128 V columns for token tile tt
                ps = opsp.tile([P, QCH], f32, tag="pps", name="pps")
                csl = slice(p * P, (p + 1) * P)
                for di in range(NDT):
                    nc.tensor.matmul(
                        ps[:, 0:P], xt_sb[:, di, tt * P : (tt + 1) * P],
                        wv_ch[:, di, csl],
                        start=(di == 0), stop=(di == NDT - 1),
                    )
                nc.vector.tensor_tensor(
                    vp_sb[tt][:, 2 * p : 2 * p + 2, 0:DK],
                    ps[:, 0:P].rearrange("q (h d) -> q h d", d=DK),
                    bv_rep[:, csl].rearrange("q (h d) -> q h d", d=DK),
                    ADD,
                )

            out_v = out[:].rearrange("(tt p) d -> p tt d", p=P)
            part_tiles = {}

            def do_OP(ch, tg):
                # 2 token tiles -> one grouped result tile -> one DMA
                res = resp.tile([P, 2, QCH], mybir.dt.float16, tag="ores", name="ores")
                for k in range(2):
                    ttk = 2 * tg + k
                    ps = opsp.tile([P, QCH], f32, tag="pps", name="pps")
                    for p in range(NPAIR):
                        nc.tensor.matmul(
                            ps[:], ob_sb[p][:, ttk, :], wo_ch[:, ch, p, :],
                            start=(p == 0), stop=(p == NPAIR - 1),
                        )
                    nc.vector.tensor_tensor(
                        res[:, k, :], ps[:], bo_rep[:, ch * QCH : (ch + 1) * QCH], ADD
                    )
                nc.sync.dma_start(
                    out_v[:, 2 * tg : 2 * tg + 2, ch * QCH : (ch + 1) * QCH], res[:]
                )

            def do_OPP(ch, tg):
                # partial out-proj (pairs 0..2) + bias, staged to SBUF so the
                # final (pair-3) contribution is all that's left for the tail
                part = partp.tile([P, 2, QCH], bf16, tag="part", name="part")
                for k in range(2):
                    ttk = 2 * tg + k
                    ps = opsp.tile([P, QCH], f32, tag="pps", name="pps")
                    for p in range(NPAIR - 1):
                        nc.tensor.matmul(
                            ps[:], ob_sb[p][:, ttk, :], wo_ch[:, ch, p, :],
                            start=(p == 0), stop=(p == NPAIR - 2),
                        )
                    nc.vector.tensor_tensor(
                        part[:, k, :], ps[:], bo_rep[:, ch * QCH : (ch + 1) * QCH], ADD
                    )
                part_tiles[(ch, tg)] = part

            def do_OPF(ch, tg):
                # tail finals: the staged partial joins the accumulation via
                # an identity matmul, and the PSUM->SBUF copies alternate
                # between Act (idle once the exp stream drains) and DVE
                part = part_tiles.pop((ch, tg))
                res = resp.tile([P, 2, QCH], mybir.dt.float16, tag="ores", name="ores")
                for k in range(2):
                    ttk = 2 * tg + k
                    ps = opsp.tile([P, QCH], f32, tag="pps", name="pps")
                    nc.tensor.matmul(
                        ps[:], ob_sb[NPAIR - 1][:, ttk, :], wo_ch[:, ch, NPAIR - 1, :],
                        start=True, stop=False,
                    )
                    nc.tensor.matmul(
                        ps[:], id_sb[:], part[:, k, :], start=False, stop=True,
                    )
                    if k == 0:
                        nc.scalar.activation(res[:, k, :], ps[:], FT.Copy)
                    else:
                        nc.vector.tensor_copy(res[:, k, :], ps[:])
                nc.sync.dma_start(
                    out_v[:, 2 * tg : 2 * tg + 2, ch * QCH : (ch + 1) * QCH], res[:]
                )

            COSTS = {"K": 4096, "Q": 4096, "V": 1024, "OP": 4096, "OPP": 3072,
                     "OPF": 1024}
            EMIT = {"K": do_K, "Q": do_Q, "V": do_V, "OP": do_OP, "OPP": do_OPP,
                    "OPF": do_OPF}

            state = {"budget": 0}
            emitted = set()
            queue = []  # ordered filler keys

            def emit_item(key):
                if key in emitted:
                    return
                emitted.add(key)
                EMIT[key[0]](*key[1:])
                state["budget"] -= COSTS[key[0]]

            def pump(margin=1 << 30):
                # never start an item that would overdraft the period budget
                # by more than `margin`: a 4096-cycle projection on a nearly
                # spent budget stacks ~2 periods of PE work ahead of the next
                # scores and stalls the exp stream
                while queue and state["budget"] > 0:
                    key = queue[0]
                    if key in emitted:
                        queue.pop(0)
                        continue
                    if COSTS[key[0]] > state["budget"] + margin:
                        break
                    queue.pop(0)
                    emit_item(key)

            # filler queue: V per (pair, tt) so early blocks only need pair 0's
            # V; K chunks get pull-emitted exactly when scores need them.
            for p in range(NPAIR):
                for ch in range(4):
                    queue.append(("K", p, ch))
                for tt in range(NKT):
                    queue.append(("V", p, tt))

            # ---------------- phase 2 machinery ----------------
            blocks = [(c, p) for p in range(NPAIR) for c in range(NQC)]
            pt_tiles = {}     # (bi, g, head) -> pt AP
            av_pending = []   # (bi, qt) in emission order
            norm_cnt = [0] * NQC

            def emit_scores(bi, g):
                c, p = blocks[bi]
                qsl = slice(c * QCH, (c + 1) * QCH)
                for head, base, tag in ((0, 0, "sgA"), (1, 64, "sgB")):
                    sg = sgp.tile([P, 2, QCH], f32, tag=tag, name=tag)
                    for j in range(2):
                        kt = 2 * g + j
                        ksl = slice(kt * P, (kt + 1) * P)
                        nc.tensor.matmul(
                            sg[:, j, :],
                            kt_f8[p][base : base + DK, :, ksl],
                            qt_f8[p][base : base + DK, :, qsl],
                            start=True, stop=True,
                            perf_mode=DR,
                            tile_position=(base, 0),
                        )
                    pt = ptp.tile([P, 2, QCH], bf16, tag=f"pt{head}", name="pt")
                    nc.scalar.activation(pt[:], sg[:], FT.Exp, scale=0.125)
                    pt_tiles[(bi, g, head)] = pt
                state["budget"] -= 1024

            def emit_chain(bi, qt):
                # AV for one query tile: per head, a 16-kt accumulation chain
                # in an exclusive PSUM bank (one open group per 2KB zero
                # region), then normalize + transpose out.
                c, p = blocks[bi]
                if qt == 0:
                    for tt in range(NKT):
                        emit_item(("V", p, tt))
                qsl = slice(qt * P, (qt + 1) * P)
                opk = opkp.tile([P, P], bf16, tag="opk", name="opk")
                for head in (0, 1):
                    acc = accp.tile([P, QCH], f32, tag=f"acc{head}", name="acc")
                    h = 2 * p + head
                    for kt in range(NKT):
                        nc.tensor.matmul(
                            acc[:, 0 : DK + 1],
                            pt_tiles[(bi, kt // 2, head)][:, kt % 2, qsl],
                            vp_sb[kt][:, h, :],
                            start=(kt == 0),
                            stop=(kt == NKT - 1),
                        )
                    rec = recp.tile([P, 1], f32, tag=f"rec{head}", name="rec")
                    nc.vector.reciprocal(rec[:], acc[:, DK : DK + 1])
                    nc.vector.tensor_tensor(
                        opk[:, head * DK : (head + 1) * DK], acc[:, 0:DK],
                        rec[:].to_broadcast((P, DK)), MUL,
                    )
                if bi == len(blocks) - 1:
                    # tail: PE transpose + DVE copy (~0.6us) instead of the
                    # ~2.3us xbar-DMA latency chain
                    tp = opsp.tile([P, P], bf16, tag="pps", name="tp")
                    nc.tensor.matmul(
                        tp[:], opk[:], id_sb[:], start=True, stop=True,
                        is_transpose=True,
                    )
                    nc.vector.tensor_copy(ob_sb[p][:, c * NQC + qt, :], tp[:])
                else:
                    nc.sync.dma_start_transpose(ob_sb[p][:, c * NQC + qt, :], opk[:])
                state["budget"] -= 2080
                if c == NQC - 1 and p == NPAIR - 1 and qt in (1, NQC - 1):
                    # last block: queue each final as soon as its own token
                    # tiles are transposed so finals overlap later chains
                    for ch in range(2):
                        queue.append(("OPF", ch, 2 * c + (0 if qt == 1 else 1)))
                if qt == NQC - 1:
                    for g in range(NG):
                        for head in (0, 1):
                            del pt_tiles[(bi, g, head)]
                    norm_cnt[c] += 1
                    if c == NQC - 1:
                        # last chunk: staged partials once pairs 0..2 done
                        if norm_cnt[c] == NPAIR - 1:
                            for ch in range(2):
                                for tg in (2 * c, 2 * c + 1):
                                    queue.append(("OPP", ch, tg))
                    elif norm_cnt[c] == NPAIR:
                        for ch in range(2):
                            for tg in (2 * c, 2 * c + 1):
                                queue.append(("OP", ch, tg))

            def drain_av():
                cap = 1 if len(av_pending) <= NQC else 2
                n = 0
                while av_pending and n < cap:
                    bi, qt = av_pending[0]
                    if exp_done[0] < (bi + 1) * NG:
                        break
                    av_pending.pop(0)
                    emit_chain(bi, qt)
                    n += 1

            exp_done = [0]

            # ---------------- prologue + main loop ----------------
            # PE p-state warmup: the clock ramps only while the engine is
            # continuously busy, so chew on a zeroed scratch tile during the
            # initial DMA wait to enter the first projections at full speed.
            warm = big.tile([P, QCH], bf16, name="warm")
            nc.vector.memset(warm[:], 0.0)
            for w in range(10):
                wps = opsp.tile([P, QCH], f32, tag="pps", name="pps")
                nc.tensor.matmul(
                    wps[:], warm[:, 0:P], warm[:], start=True, stop=True,
                )

            emit_item(("Q", 0, 0))
            emit_item(("K", 0, 0))
            state["budget"] = PROLOGUE_BUDGET

            for bi, (c, p) in enumerate(blocks):
                emit_item(("Q", c, p))
                for g in range(NG):
                    emit_item(("K", p, g // 2))
                    emit_scores(bi, g)
                    # pull the next K chunk right after this period's scores,
                    # two periods ahead of the scores that will need it
                    emit_item(("K", p, min(NG // 2 - 1, g // 2 + 1)))
                    exp_done[0] += 1
                    # pre-pull the next block's projections (spread across two
                    # early periods) so its first scores are never gated on a
                    # just-emitted K/Q and the burst never delays this block's
                    # own next scores by more than a period
                    if bi + 1 < len(blocks):
                        cn, pn = blocks[bi + 1]
                        if g == 0:
                            emit_item(("K", pn, 0))
                        elif g == 2:
                            emit_item(("Q", cn, pn))
                    drain_av()
                    state["budget"] += SLACK_CYC if bi else SLACK_CYC - 1600
                    pump()
                    if state["budget"] > BUDGET_CAP:
                        state["budget"] = BUDGET_CAP
                for qt in range(NQC):
                    av_pending.append((bi, qt))

            # tail: drain chains with the pump interleaved so out-proj
            # finals start as soon as their token tiles are transposed
            while av_pending:
                bi, qt = av_pending.pop(0)
                state["budget"] = 4000
                emit_chain(bi, qt)
                pump(margin=1 << 30)
            state["budget"] = 1 << 30
            pump(margin=1 << 30)

    nc.compile()
    return nc


def _prep_inputs(x, Wq, bq, Wk, bk, Wv, bv, Wo, bo):
    """Shard + lay out inputs for the 8 cores (batch x head-group)."""
    x = np.asarray(x, dtype=np.float32)
    to_bf = lambda a: np.ascontiguousarray(a).astype(ml_dtypes.bfloat16)
    Wq, Wk, Wv, Wo = (np.asarray(w, np.float32) for w in (Wq, Wk, Wv, Wo))
    bq, bv, bo = (np.asarray(v, np.float32) for v in (bq, bv, bo))
    bo_half = np.ascontiguousarray((bo * 0.5).reshape(1, D)).astype(
        ml_dtypes.bfloat16
    )
    xTb = [to_bf(x[b].T) for b in range(B)]
    in_maps = []
    for core in range(NCORES):
        b, hg = core // 2, core % 2
        csl = slice(hg * DC, (hg + 1) * DC)

        def tile_qk(W):
            # [D, DC] -> [p, dt, a, m]
            return to_bf(
                W[:, csl].reshape(NDT, P, NHT, P).transpose(1, 2, 0, 3)
            )

        in_maps.append(
            {
                "xT": xTb[b],
                "Wq": tile_qk(Wq),
                "Wk": tile_qk(Wk),
                "Wv": to_bf(Wv[:, csl].reshape(NDT, P, DC).transpose(1, 0, 2)),
                "Wo": to_bf(
                    Wo[csl, :].reshape(NHT, P, 2, QCH).transpose(1, 2, 0, 3)
                ),
                "bqp": np.ascontiguousarray(bq[csl].reshape(NHT, P).T),
                "bv": np.ascontiguousarray(bv[csl].reshape(1, DC)),
                "ident": np.eye(P, dtype=ml_dtypes.bfloat16),
                "bo": bo_half,
            }
        )
    return in_maps


def kernel(x, Wq, bq, Wk, bk, Wv, bv, Wo, bo):
    if "nc" not in _CACHE:
        _CACHE["nc"] = build_kernel()
    nc = _CACHE["nc"]
    in_maps = _prep_inputs(x, Wq, bq, Wk, bk, Wv, bv, Wo, bo)
    res = run_bass_kernel_spmd(nc, in_maps, list(range(NCORES)))
    out = np.empty((B, T, D), dtype=np.float32)
    for b in range(B):
        out[b] = res.results[2 * b]["out"].astype(np.float32) + res.results[
            2 * b + 1
        ]["out"].astype(np.float32)
    return out


# revision 45
# speedup vs baseline: 1.0006x; 1.0003x over previous
import os
import sys

for _p in ("/opt/trn_rl_repo", os.path.expanduser("~/.axon_site/_ro/trn_rl_repo")):
    if os.path.isdir(_p) and _p not in sys.path:
        sys.path.insert(0, _p)

import numpy as np
import ml_dtypes

import concourse.bass as bass
from concourse import bacc
import concourse.tile as tile
import concourse.mybir as mybir
from concourse.bass_utils import run_bass_kernel_spmd

# Problem shape (hardcoded per contract)
B, T, D, H, DK = 4, 2048, 1024, 16, 64
NCORES = 8

# Sharding: core = (batch b, head-group hg). Each core handles 8 heads of one
# batch over the full sequence, row-shards W_o, and the host sums the two
# partial outputs per batch (the "all-reduce" of the tensor-parallel scheme).
HC = H // 2       # 8 heads per core
DC = HC * DK      # 512 hidden dims per core

P = 128
NDT = D // P      # 8 din tiles
NHT = DC // P     # 4 dout tiles (= head pairs) for this core's heads
NKT = T // P      # 16 key-token tiles
NPAIR = HC // 2   # 4 head pairs (pair p <-> dout tile p)
QCH = 512         # free-dim chunk per q-chunk
NQC = T // QCH    # 4 q-chunks
NG = NKT // 2     # 8 kt-groups (2 kt each) per (chunk, pair) block

bf16 = mybir.dt.bfloat16
fp8 = mybir.dt.float8e4
f32 = mybir.dt.float32
FT = mybir.ActivationFunctionType
ADD = mybir.AluOpType.add
MUL = mybir.AluOpType.mult
DR = mybir.MatmulPerfMode.DoubleRow

_CACHE = {}

# Software-pipeline tuning: PE cycle budget granted per exp-period. The Act
# engine (exp stream) paces the kernel at ~2076 ns / period (~4980 PE cycles
# at 2.4 GHz); scores, AV chains and filler items all charge against it.
SLACK_CYC = 4600
BUDGET_CAP = 1500
PROLOGUE_BUDGET = 8000


def build_kernel():
    nc = bacc.Bacc("TRN2", target_bir_lowering=False, debug=False, num_devices=1)

    # Per-core inputs, pre-tiled on the host so every DMA is one contiguous
    # transfer (HWDGE dispatch is ~625ns per DMA, so fewer/bigger is better):
    # Wq/Wk: [p, dt, a, m] = W[a*128+p, dt*128+m]; Wv: [p, a, m] = W[a*128+p, m]
    # Wo: [p, ch, a, m] = Wo_shard[a*128+p, ch*512+m]
    xT = nc.dram_tensor("xT", [D, T], bf16, kind="ExternalInput")  # x[b].T
    Wq = nc.dram_tensor("Wq", [P, NHT, NDT, P], bf16, kind="ExternalInput")
    Wk = nc.dram_tensor("Wk", [P, NHT, NDT, P], bf16, kind="ExternalInput")
    Wv = nc.dram_tensor("Wv", [P, NDT, DC], bf16, kind="ExternalInput")
    Wo = nc.dram_tensor("Wo", [P, 2, NHT, QCH], bf16, kind="ExternalInput")
    # bq pre-striped on host to [128, NHT] (col t = bias[t*128:(t+1)*128]).
    # The K bias is dropped entirely: softmax over keys is invariant to the
    # (q+bq)@bk term, so scores use (q+bq)@k with k = x@Wk (no bias).
    bqp = nc.dram_tensor("bqp", [P, NHT], f32, kind="ExternalInput")
    bv = nc.dram_tensor("bv", [1, DC], f32, kind="ExternalInput")
    ident = nc.dram_tensor("ident", [P, P], bf16, kind="ExternalInput")
    bo = nc.dram_tensor("bo", [1, D], bf16, kind="ExternalInput")  # pre-halved
    out = nc.dram_tensor("out", [T, D], mybir.dt.float16, kind="ExternalOutput")

    xTv = xT[:].rearrange("(a p) t -> p a t", p=P)  # [128, NDT, T] view

    with tile.TileContext(nc) as tc:
        with (
            tc.tile_pool(name="big", bufs=1) as big,
            tc.tile_pool(name="ptp", bufs=12) as ptp,
            tc.tile_pool(name="opk", bufs=8) as opkp,
            tc.tile_pool(name="rec", bufs=2) as recp,
            tc.tile_pool(name="part", bufs=4) as partp,
            tc.tile_pool(name="res", bufs=4) as resp,
            tc.tile_pool(name="sg", bufs=1, space="PSUM") as sgp,
            tc.tile_pool(name="acc", bufs=1, space="PSUM") as accp,
            tc.tile_pool(name="ops", bufs=2, space="PSUM") as opsp,
        ):
            # ---------------- static SBUF tiles + input DMA ----------------
            wk_w = big.tile([P, NHT, NDT, P], bf16, name="wk_w")
            wq_w = big.tile([P, NHT, NDT, P], bf16, name="wq_w")
            xt_sb = big.tile([P, NDT, T], bf16, name="xt_sb")
            bq_sb = big.tile([P, NHT], f32, name="bq_sb")
            bv_rep = big.tile([P, DC], f32, name="bv_rep")
            bo_rep = big.tile([P, D], bf16, name="bo_rep")
            wv_ch = big.tile([P, NDT, DC], bf16, name="wv_ch")
            wo_ch = big.tile([P, 2, NHT, QCH], bf16, name="wo_ch")

            # fp8 Q/K for DoubleRow scores. Layout [p, i, t]: partition p<64 =
            # head-A dims, p>=64 = head-B dims; i is the DoubleRow k-tile dim
            # (tile 1 kept zero so contraction-64 runs at 0.5 cyc/row).
            id_sb = big.tile([P, P], bf16, name="id_sb")
            kt_f8 = [big.tile([P, 2, T], fp8, name=f"ktf{p}") for p in range(NPAIR)]
            qt_f8 = [big.tile([P, 2, T], fp8, name=f"qtf{p}") for p in range(NPAIR)]
            # V with bias, ones column at [:, h, 64] for softmax denominators
            vp_sb = [big.tile([P, HC, DK + 1], bf16, name=f"vp{t}") for t in range(NKT)]
            # attention outputs, transposed: [din of pair, tt, token]
            ob_sb = [big.tile([P, NKT, P], bf16, name=f"ob{p}") for p in range(NPAIR)]

            # input loads, priority order, all on SP/HWDGE. The first
            # scores' critical path is x[chunk0] + Wk[pair0] -> Wq[pair0];
            # chunk0 is split so the K matmuls can start on the first half.
            nc.sync.dma_start(xt_sb[:, 0 : NDT // 2, 0:QCH], xTv[:, 0 : NDT // 2, 0:QCH])
            nc.sync.dma_start(wk_w[:, 0], Wk[:, 0])
            nc.sync.dma_start(xt_sb[:, NDT // 2 :, 0:QCH], xTv[:, NDT // 2 :, 0:QCH])
            nc.sync.dma_start(wq_w[:, 0], Wq[:, 0])
            nc.sync.dma_start(bq_sb[:], bqp[:])
            nc.sync.dma_start(xt_sb[:, :, QCH : 2 * QCH], xTv[:, :, QCH : 2 * QCH])
            nc.sync.dma_start(bv_rep[:], bv[:].to_broadcast((P, DC)))
            nc.sync.dma_start(wv_ch[:], Wv[:])
            nc.sync.dma_start(xt_sb[:, :, 2 * QCH : 3 * QCH], xTv[:, :, 2 * QCH : 3 * QCH])
            nc.sync.dma_start(wk_w[:, 1:NHT], Wk[:, 1:NHT])
            nc.sync.dma_start(xt_sb[:, :, 3 * QCH :], xTv[:, :, 3 * QCH :])
            nc.sync.dma_start(wq_w[:, 1:NHT], Wq[:, 1:NHT])
            nc.sync.dma_start(wo_ch[:], Wo[:])
            nc.sync.dma_start(bo_rep[:], bo[:].to_broadcast((P, D)))
            nc.sync.dma_start(id_sb[:], ident[:])

            # memsets on Pool (otherwise idle): DoubleRow zero-tiles for pair 0
            # first (needed by the first scores), then denominators' ones
            # column, then the remaining pairs.
            nc.gpsimd.memset(kt_f8[0][:, 1, :], 0.0)
            nc.gpsimd.memset(qt_f8[0][:, 1, :], 0.0)
            for t in range(NKT):
                nc.gpsimd.memset(vp_sb[t][:, :, DK : DK + 1], 1.0)
            for p in range(1, NPAIR):
                nc.gpsimd.memset(kt_f8[p][:, 1, :], 0.0)
                nc.gpsimd.memset(qt_f8[p][:, 1, :], 0.0)

            # ---------------- work items (PE filler) ----------------
            kconvs = [0]

            def do_K(p, ch):
                sl = slice(ch * QCH, (ch + 1) * QCH)
                ps = opsp.tile([P, QCH], f32, tag="pps", name="pps")
                for di in range(NDT):
                    nc.tensor.matmul(
                        ps[:], wk_w[:, p, di, :], xt_sb[:, di, sl],
                        start=(di == 0), stop=(di == NDT - 1),
                    )
                if kconvs[0] == 0:
                    # first K: convert in halves so the first scores (keys
                    # 0:256) only wait for the first half
                    h = QCH // 2
                    nc.vector.tensor_copy(kt_f8[p][:, 0, ch * QCH : ch * QCH + h], ps[:, 0:h])
                    nc.vector.tensor_copy(kt_f8[p][:, 0, ch * QCH + h : (ch + 1) * QCH], ps[:, h:])
                else:
                    nc.vector.tensor_copy(kt_f8[p][:, 0, sl], ps[:])
                kconvs[0] += 1

            def do_Q(c, p):
                sl = slice(c * QCH, (c + 1) * QCH)
                ps = opsp.tile([P, QCH], f32, tag="pps", name="pps")
                for di in range(NDT):
                    nc.tensor.matmul(
                        ps[:], wq_w[:, p, di, :], xt_sb[:, di, sl],
                        start=(di == 0), stop=(di == NDT - 1),
                    )
                nc.vector.tensor_tensor(
                    qt_f8[p][:, 0, sl], ps[:],
                    bq_sb[:, p : p + 1].to_broadcast((P, QCH)), ADD,
                )

            def do_V(p, tt):
                # one pair's 128 V columns for token tile tt
                ps = opsp.tile([P, QCH], f32, tag="pps", name="pps")
                csl = slice(p * P, (p + 1) * P)
                for di in range(NDT):
                    nc.tensor.matmul(
                        ps[:, 0:P], xt_sb[:, di, tt * P : (tt + 1) * P],
                        wv_ch[:, di, csl],
                        start=(di == 0), stop=(di == NDT - 1),
                    )
                nc.vector.tensor_tensor(
                    vp_sb[tt][:, 2 * p : 2 * p + 2, 0:DK],
                    ps[:, 0:P].rearrange("q (h d) -> q h d", d=DK),
                    bv_rep[:, csl].rearrange("q (h d) -> q h d", d=DK),
                    ADD,
                )

            out_v = out[:].rearrange("(tt p) d -> p tt d", p=P)
            part_tiles = {}

            def do_OP(ch, tg):
                # 2 token tiles -> one grouped result tile -> one DMA
                res = resp.tile([P, 2, QCH], mybir.dt.float16, tag="ores", name="ores")
                for k in range(2):
                    ttk = 2 * tg + k
                    ps = opsp.tile([P, QCH], f32, tag="pps", name="pps")
                    for p in range(NPAIR):
                        nc.tensor.matmul(
                            ps[:], ob_sb[p][:, ttk, :], wo_ch[:, ch, p, :],
                            start=(p == 0), stop=(p == NPAIR - 1),
                        )
                    nc.vector.tensor_tensor(
                        res[:, k, :], ps[:], bo_rep[:, ch * QCH : (ch + 1) * QCH], ADD
                    )
                nc.sync.dma_start(
                    out_v[:, 2 * tg : 2 * tg + 2, ch * QCH : (ch + 1) * QCH], res[:]
                )

            def do_OPP(ch, tg):
                # partial out-proj (pairs 0..2) + bias, staged to SBUF so the
                # final (pair-3) contribution is all that's left for the tail
                part = partp.tile([P, 2, QCH], bf16, tag="part", name="part")
                for k in range(2):
                    ttk = 2 * tg + k
                    ps = opsp.tile([P, QCH], f32, tag="pps", name="pps")
                    for p in range(NPAIR - 1):
                        nc.tensor.matmul(
                            ps[:], ob_sb[p][:, ttk, :], wo_ch[:, ch, p, :],
                            start=(p == 0), stop=(p == NPAIR - 2),
                        )
                    nc.vector.tensor_tensor(
                        part[:, k, :], ps[:], bo_rep[:, ch * QCH : (ch + 1) * QCH], ADD
                    )
                part_tiles[(ch, tg)] = part

            def do_OPF(ch, tg):
                # tail finals: the staged partial joins the accumulation via
                # an identity matmul, and the PSUM->SBUF copies alternate
                # between Act (idle once the exp stream drains) and DVE
                part = part_tiles.pop((ch, tg))
                res = resp.tile([P, 2, QCH], mybir.dt.float16, tag="ores", name="ores")
                for k in range(2):
                    ttk = 2 * tg + k
                    ps = opsp.tile([P, QCH], f32, tag="pps", name="pps")
                    nc.tensor.matmul(
                        ps[:], ob_sb[NPAIR - 1][:, ttk, :], wo_ch[:, ch, NPAIR - 1, :],
                        start=True, stop=False,
                    )
                    nc.tensor.matmul(
                        ps[:], id_sb[:], part[:, k, :], start=False, stop=True,
                    )
                    if k == 0:
                        nc.scalar.activation(res[:, k, :], ps[:], FT.Copy)
                    else:
                        nc.vector.tensor_copy(res[:, k, :], ps[:])
                nc.sync.dma_start(
                    out_v[:, 2 * tg : 2 * tg + 2, ch * QCH : (ch + 1) * QCH], res[:]
                )

            COSTS = {"K": 4096, "Q": 4096, "V": 1024, "OP": 4096, "OPP": 3072,
                     "OPF": 1024}
            EMIT = {"K": do_K, "Q": do_Q, "V": do_V, "OP": do_OP, "OPP": do_OPP,
                    "OPF": do_OPF}

            state = {"budget": 0}
            emitted = set()
            queue = []  # ordered filler keys

            def emit_item(key):
                if key in emitted:
                    return
                emitted.add(key)
                EMIT[key[0]](*key[1:])
                state["budget"] -= COSTS[key[0]]

            def pump(margin=1 << 30):
                # never start an item that would overdraft the period budget
                # by more than `margin`: a 4096-cycle projection on a nearly
                # spent budget stacks ~2 periods of PE work ahead of the next
                # scores and stalls the exp stream
                while queue and state["budget"] > 0:
                    key = queue[0]
                    if key in emitted:
                        queue.pop(0)
                        continue
                    if COSTS[key[0]] > state["budget"] + margin:
                        break
                    queue.pop(0)
                    emit_item(key)

            # filler queue: V per (pair, tt) so early blocks only need pair 0's
            # V; K chunks get pull-emitted exactly when scores need them.
            for p in range(NPAIR):
                for ch in range(4):
                    queue.append(("K", p, ch))
                for tt in range(NKT):
                    queue.append(("V", p, tt))

            # ---------------- phase 2 machinery ----------------
            blocks = [(c, p) for p in range(NPAIR) for c in range(NQC)]
            pt_tiles = {}     # (bi, g, head) -> pt AP
            av_pending = []   # (bi, qt) in emission order
            norm_cnt = [0] * NQC

            def emit_scores(bi, g):
                c, p = blocks[bi]
                qsl = slice(c * QCH, (c + 1) * QCH)
                for head, base, tag in ((0, 0, "sgA"), (1, 64, "sgB")):
                    sg = sgp.tile([P, 2, QCH], f32, tag=tag, name=tag)
                    for j in range(2):
                        kt = 2 * g + j
                        ksl = slice(kt * P, (kt + 1) * P)
                        nc.tensor.matmul(
                            sg[:, j, :],
                            kt_f8[p][base : base + DK, :, ksl],
                            qt_f8[p][base : base + DK, :, qsl],
                            start=True, stop=True,
                            perf_mode=DR,
                            tile_position=(base, 0),
                        )
                    pt = ptp.tile([P, 2, QCH], bf16, tag=f"pt{head}", name="pt")
                    nc.scalar.activation(pt[:], sg[:], FT.Exp, scale=0.125)
                    pt_tiles[(bi, g, head)] = pt
                state["budget"] -= 1024

            def emit_chain(bi, qt):
                # AV for one query tile: per head, a 16-kt accumulation chain
                # in an exclusive PSUM bank (one open group per 2KB zero
                # region), then normalize + transpose out.
                c, p = blocks[bi]
                if qt == 0:
                    for tt in range(NKT):
                        emit_item(("V", p, tt))
                qsl = slice(qt * P, (qt + 1) * P)
                opk = opkp.tile([P, P], bf16, tag="opk", name="opk")
                for head in (0, 1):
                    acc = accp.tile([P, QCH], f32, tag=f"acc{head}", name="acc")
                    h = 2 * p + head
                    for kt in range(NKT):
                        nc.tensor.matmul(
                            acc[:, 0 : DK + 1],
                            pt_tiles[(bi, kt // 2, head)][:, kt % 2, qsl],
                            vp_sb[kt][:, h, :],
                            start=(kt == 0),
                            stop=(kt == NKT - 1),
                        )
                    rec = recp.tile([P, 1], f32, tag=f"rec{head}", name="rec")
                    nc.vector.reciprocal(rec[:], acc[:, DK : DK + 1])
                    nc.vector.tensor_tensor(
                        opk[:, head * DK : (head + 1) * DK], acc[:, 0:DK],
                        rec[:].to_broadcast((P, DK)), MUL,
                    )
                if bi == len(blocks) - 1:
                    # tail: PE transpose + DVE copy (~0.6us) instead of the
                    # ~2.3us xbar-DMA latency chain
                    tp = opsp.tile([P, P], bf16, tag="pps", name="tp")
                    nc.tensor.matmul(
                        tp[:], opk[:], id_sb[:], start=True, stop=True,
                        is_transpose=True,
                    )
                    nc.vector.tensor_copy(ob_sb[p][:, c * NQC + qt, :], tp[:])
                else:
                    nc.sync.dma_start_transpose(ob_sb[p][:, c * NQC + qt, :], opk[:])
                state["budget"] -= 2080
                if c == NQC - 1 and p == NPAIR - 1 and qt in (1, NQC - 1):
                    # last block: queue each final as soon as its own token
                    # tiles are transposed so finals overlap later chains
                    for ch in range(2):
                        queue.append(("OPF", ch, 2 * c + (0 if qt == 1 else 1)))
                if qt == NQC - 1:
                    for g in range(NG):
                        for head in (0, 1):
                            del pt_tiles[(bi, g, head)]
                    norm_cnt[c] += 1
                    if c == NQC - 1:
                        # last chunk: staged partials once pairs 0..2 done
                        if norm_cnt[c] == NPAIR - 1:
                            for ch in range(2):
                                for tg in (2 * c, 2 * c + 1):
                                    queue.append(("OPP", ch, tg))
                    elif norm_cnt[c] == NPAIR:
                        for ch in range(2):
                            for tg in (2 * c, 2 * c + 1):
                                queue.append(("OP", ch, tg))

            def drain_av():
                cap = 1 if len(av_pending) <= NQC else 2
                n = 0
                while av_pending and n < cap:
                    bi, qt = av_pending[0]
                    if exp_done[0] < (bi + 1) * NG:
                        break
                    av_pending.pop(0)
                    emit_chain(bi, qt)
                    n += 1

            exp_done = [0]

            # ---------------- prologue + main loop ----------------
            # PE p-state warmup: the clock ramps only while the engine is
            # continuously busy, so chew on a zeroed scratch tile during the
            # initial DMA wait to enter the first projections at full speed.
            warm = big.tile([P, QCH], bf16, name="warm")
            nc.vector.memset(warm[:], 0.0)
            for w in range(10):
                wps = opsp.tile([P, QCH], f32, tag="pps", name="pps")
                nc.tensor.matmul(
                    wps[:], warm[:, 0:P], warm[:], start=True, stop=True,
                )

            emit_item(("Q", 0, 0))
            emit_item(("K", 0, 0))
            state["budget"] = PROLOGUE_BUDGET

            for bi, (c, p) in enumerate(blocks):
                emit_item(("Q", c, p))
                for g in range(NG):
                    emit_item(("K", p, g // 2))
                    emit_scores(bi, g)
                    # pull the next K chunk right after this period's scores,
                    # two periods ahead of the scores that will need it
                    emit_item(("K", p, min(NG // 2 - 1, g // 2 + 1)))
                    exp_done[0] += 1
                    # pre-pull the next block's projections (spread across two
                    # early periods) so its first scores are never gated on a
                    # just-emitted K/Q and the burst never delays this block's
                    # own next scores by more than a period
                    if bi + 1 < len(blocks):
                        cn, pn = blocks[bi + 1]
                        if g == 0:
                            emit_item(("K", pn, 0))
                        elif g == 2:
                            emit_item(("Q", cn, pn))
                    drain_av()
                    state["budget"] += SLACK_CYC if bi else SLACK_CYC - 1600
                    pump()
                    if state["budget"] > BUDGET_CAP:
                        state["budget"] = BUDGET_CAP
                for qt in range(NQC):
                    av_pending.append((bi, qt))

            # tail: drain chains with the pump interleaved so out-proj
            # finals start as soon as their token tiles are transposed
            while av_pending:
                bi, qt = av_pending.pop(0)
                state["budget"] = 4000
                emit_chain(bi, qt)
                pump(margin=1 << 30)
            state["budget"] = 1 << 30
            pump(margin=1 << 30)

    nc.compile()
    return nc


def _prep_inputs(x, Wq, bq, Wk, bk, Wv, bv, Wo, bo):
    """Shard + lay out inputs for the 8 cores (batch x head-group)."""
    x = np.asarray(x, dtype=np.float32)
    to_bf = lambda a: np.ascontiguousarray(a).astype(ml_dtypes.bfloat16)
    Wq, Wk, Wv, Wo = (np.asarray(w, np.float32) for w in (Wq, Wk, Wv, Wo))
    bq, bv, bo = (np.asarray(v, np.float32) for v in (bq, bv, bo))
    bo_half = np.ascontiguousarray((bo * 0.5).reshape(1, D)).astype(
        ml_dtypes.bfloat16
    )
    xTb = [to_bf(x[b].T) for b in range(B)]
    in_maps = []
    for core in range(NCORES):
        b, hg = core // 2, core % 2
        csl = slice(hg * DC, (hg + 1) * DC)

        def tile_qk(W):
            # [D, DC] -> [p, dt, a, m]
            return to_bf(
                W[:, csl].reshape(NDT, P, NHT, P).transpose(1, 2, 0, 3)
            )

        in_maps.append(
            {
                "xT": xTb[b],
                "Wq": tile_qk(Wq),
                "Wk": tile_qk(Wk),
                "Wv": to_bf(Wv[:, csl].reshape(NDT, P, DC).transpose(1, 0, 2)),
                "Wo": to_bf(
                    Wo[csl, :].reshape(NHT, P, 2, QCH).transpose(1, 2, 0, 3)
                ),
                "bqp": np.ascontiguousarray(bq[csl].reshape(NHT, P).T),
                "bv": np.ascontiguousarray(bv[csl].reshape(1, DC)),
                "ident": np.eye(P, dtype=ml_dtypes.bfloat16),
                "bo": bo_half,
            }
        )
    return in_maps


def kernel(x, Wq, bq, Wk, bk, Wv, bv, Wo, bo):
    if "nc" not in _CACHE:
        _CACHE["nc"] = build_kernel()
    nc = _CACHE["nc"]
    in_maps = _prep_inputs(x, Wq, bq, Wk, bk, Wv, bv, Wo, bo)
    res = run_bass_kernel_spmd(nc, in_maps, list(range(NCORES)))
    out = np.empty((B, T, D), dtype=np.float32)
    for b in range(B):
        out[b] = res.results[2 * b]["out"].astype(np.float32) + res.results[
            2 * b + 1
        ]["out"].astype(np.float32)
    return out


# revision 46
# speedup vs baseline: 1.0017x; 1.0011x over previous
import os
import sys

for _p in ("/opt/trn_rl_repo", os.path.expanduser("~/.axon_site/_ro/trn_rl_repo")):
    if os.path.isdir(_p) and _p not in sys.path:
        sys.path.insert(0, _p)

import numpy as np
import ml_dtypes

import concourse.bass as bass
from concourse import bacc
import concourse.tile as tile
import concourse.mybir as mybir
from concourse.bass_utils import run_bass_kernel_spmd

# Problem shape (hardcoded per contract)
B, T, D, H, DK = 4, 2048, 1024, 16, 64
NCORES = 8

# Sharding: core = (batch b, head-group hg). Each core handles 8 heads of one
# batch over the full sequence, row-shards W_o, and the host sums the two
# partial outputs per batch (the "all-reduce" of the tensor-parallel scheme).
HC = H // 2       # 8 heads per core
DC = HC * DK      # 512 hidden dims per core

P = 128
NDT = D // P      # 8 din tiles
NHT = DC // P     # 4 dout tiles (= head pairs) for this core's heads
NKT = T // P      # 16 key-token tiles
NPAIR = HC // 2   # 4 head pairs (pair p <-> dout tile p)
QCH = 512         # free-dim chunk per q-chunk
NQC = T // QCH    # 4 q-chunks
NG = NKT // 2     # 8 kt-groups (2 kt each) per (chunk, pair) block

bf16 = mybir.dt.bfloat16
fp8 = mybir.dt.float8e4
f32 = mybir.dt.float32
FT = mybir.ActivationFunctionType
ADD = mybir.AluOpType.add
MUL = mybir.AluOpType.mult
DR = mybir.MatmulPerfMode.DoubleRow

_CACHE = {}

# Software-pipeline tuning: PE cycle budget granted per exp-period. The Act
# engine (exp stream) paces the kernel at ~2076 ns / period (~4980 PE cycles
# at 2.4 GHz); scores, AV chains and filler items all charge against it.
SLACK_CYC = 4600
BUDGET_CAP = 1500
PROLOGUE_BUDGET = 8000


def build_kernel():
    nc = bacc.Bacc("TRN2", target_bir_lowering=False, debug=False, num_devices=1)

    # Per-core inputs, pre-tiled on the host so every DMA is one contiguous
    # transfer (HWDGE dispatch is ~625ns per DMA, so fewer/bigger is better):
    # Wq/Wk: [p, dt, a, m] = W[a*128+p, dt*128+m]; Wv: [p, a, m] = W[a*128+p, m]
    # Wo: [p, ch, a, m] = Wo_shard[a*128+p, ch*512+m]
    xT = nc.dram_tensor("xT", [D, T], bf16, kind="ExternalInput")  # x[b].T
    Wq = nc.dram_tensor("Wq", [P, NHT, NDT, P], bf16, kind="ExternalInput")
    Wk = nc.dram_tensor("Wk", [P, NHT, NDT, P], bf16, kind="ExternalInput")
    Wv = nc.dram_tensor("Wv", [P, NDT, DC], bf16, kind="ExternalInput")
    Wo = nc.dram_tensor("Wo", [P, 2, NHT, QCH], bf16, kind="ExternalInput")
    # bq pre-striped on host to [128, NHT] (col t = bias[t*128:(t+1)*128]).
    # The K bias is dropped entirely: softmax over keys is invariant to the
    # (q+bq)@bk term, so scores use (q+bq)@k with k = x@Wk (no bias).
    bqp = nc.dram_tensor("bqp", [P, NHT], f32, kind="ExternalInput")
    bv = nc.dram_tensor("bv", [1, DC], f32, kind="ExternalInput")
    ident = nc.dram_tensor("ident", [P, P], bf16, kind="ExternalInput")
    bo = nc.dram_tensor("bo", [1, D], bf16, kind="ExternalInput")  # pre-halved
    out = nc.dram_tensor("out", [T, D], mybir.dt.float16, kind="ExternalOutput")

    xTv = xT[:].rearrange("(a p) t -> p a t", p=P)  # [128, NDT, T] view

    with tile.TileContext(nc) as tc:
        with (
            tc.tile_pool(name="big", bufs=1) as big,
            tc.tile_pool(name="ptp", bufs=12) as ptp,
            tc.tile_pool(name="opk", bufs=8) as opkp,
            tc.tile_pool(name="rec", bufs=2) as recp,
            tc.tile_pool(name="part", bufs=4) as partp,
            tc.tile_pool(name="res", bufs=4) as resp,
            tc.tile_pool(name="sg", bufs=1, space="PSUM") as sgp,
            tc.tile_pool(name="acc", bufs=1, space="PSUM") as accp,
            tc.tile_pool(name="ops", bufs=2, space="PSUM") as opsp,
        ):
            # ---------------- static SBUF tiles + input DMA ----------------
            wk_w = big.tile([P, NHT, NDT, P], bf16, name="wk_w")
            wq_w = big.tile([P, NHT, NDT, P], bf16, name="wq_w")
            xt_sb = big.tile([P, NDT, T], bf16, name="xt_sb")
            bq_sb = big.tile([P, NHT], f32, name="bq_sb")
            bv_rep = big.tile([P, DC], f32, name="bv_rep")
            bo_rep = big.tile([P, D], bf16, name="bo_rep")
            wv_ch = big.tile([P, NDT, DC], bf16, name="wv_ch")
            wo_ch = big.tile([P, 2, NHT, QCH], bf16, name="wo_ch")

            # fp8 Q/K for DoubleRow scores. Layout [p, i, t]: partition p<64 =
            # head-A dims, p>=64 = head-B dims; i is the DoubleRow k-tile dim
            # (tile 1 kept zero so contraction-64 runs at 0.5 cyc/row).
            id_sb = big.tile([P, P], bf16, name="id_sb")
            kt_f8 = [big.tile([P, 2, T], fp8, name=f"ktf{p}") for p in range(NPAIR)]
            qt_f8 = [big.tile([P, 2, T], fp8, name=f"qtf{p}") for p in range(NPAIR)]
            # V with bias, ones column at [:, h, 64] for softmax denominators
            vp_sb = [big.tile([P, HC, DK + 1], bf16, name=f"vp{t}") for t in range(NKT)]
            # attention outputs, transposed: [din of pair, tt, token]
            ob_sb = [big.tile([P, NKT, P], bf16, name=f"ob{p}") for p in range(NPAIR)]

            # input loads, priority order, all on SP/HWDGE. The first
            # scores' critical path is x[chunk0] + Wk[pair0] -> Wq[pair0];
            # chunk0 is split so the K matmuls can start on the first half.
            nc.sync.dma_start(xt_sb[:, 0 : NDT // 2, 0:QCH], xTv[:, 0 : NDT // 2, 0:QCH])
            nc.sync.dma_start(wk_w[:, 0], Wk[:, 0])
            nc.sync.dma_start(xt_sb[:, NDT // 2 :, 0:QCH], xTv[:, NDT // 2 :, 0:QCH])
            nc.sync.dma_start(wq_w[:, 0], Wq[:, 0])
            nc.sync.dma_start(bq_sb[:], bqp[:])
            nc.sync.dma_start(xt_sb[:, :, QCH : 2 * QCH], xTv[:, :, QCH : 2 * QCH])
            nc.sync.dma_start(bv_rep[:], bv[:].to_broadcast((P, DC)))
            nc.sync.dma_start(wv_ch[:], Wv[:])
            nc.sync.dma_start(xt_sb[:, :, 2 * QCH : 3 * QCH], xTv[:, :, 2 * QCH : 3 * QCH])
            nc.sync.dma_start(wk_w[:, 1:NHT], Wk[:, 1:NHT])
            nc.sync.dma_start(xt_sb[:, :, 3 * QCH :], xTv[:, :, 3 * QCH :])
            nc.sync.dma_start(wq_w[:, 1:NHT], Wq[:, 1:NHT])
            nc.sync.dma_start(wo_ch[:], Wo[:])
            nc.sync.dma_start(bo_rep[:], bo[:].to_broadcast((P, D)))
            nc.sync.dma_start(id_sb[:], ident[:])

            # memsets on Pool (otherwise idle): DoubleRow zero-tiles for pair 0
            # first (needed by the first scores), then denominators' ones
            # column, then the remaining pairs.
            nc.gpsimd.memset(kt_f8[0][:, 1, :], 0.0)
            nc.gpsimd.memset(qt_f8[0][:, 1, :], 0.0)
            for t in range(NKT):
                nc.gpsimd.memset(vp_sb[t][:, :, DK : DK + 1], 1.0)
            for p in range(1, NPAIR):
                nc.gpsimd.memset(kt_f8[p][:, 1, :], 0.0)
                nc.gpsimd.memset(qt_f8[p][:, 1, :], 0.0)

            # ---------------- work items (PE filler) ----------------
            kconvs = [0]

            def do_K(p, ch):
                sl = slice(ch * QCH, (ch + 1) * QCH)
                ps = opsp.tile([P, QCH], f32, tag="pps", name="pps")
                for di in range(NDT):
                    nc.tensor.matmul(
                        ps[:], wk_w[:, p, di, :], xt_sb[:, di, sl],
                        start=(di == 0), stop=(di == NDT - 1),
                    )
                if kconvs[0] == 0:
                    # first K: convert in halves so the first scores (keys
                    # 0:256) only wait for the first half
                    h = QCH // 2
                    nc.vector.tensor_copy(kt_f8[p][:, 0, ch * QCH : ch * QCH + h], ps[:, 0:h])
                    nc.vector.tensor_copy(kt_f8[p][:, 0, ch * QCH + h : (ch + 1) * QCH], ps[:, h:])
                else:
                    nc.vector.tensor_copy(kt_f8[p][:, 0, sl], ps[:])
                kconvs[0] += 1

            def do_Q(c, p):
                sl = slice(c * QCH, (c + 1) * QCH)
                ps = opsp.tile([P, QCH], f32, tag="pps", name="pps")
                for di in range(NDT):
                    nc.tensor.matmul(
                        ps[:], wq_w[:, p, di, :], xt_sb[:, di, sl],
                        start=(di == 0), stop=(di == NDT - 1),
                    )
                nc.vector.tensor_tensor(
                    qt_f8[p][:, 0, sl], ps[:],
                    bq_sb[:, p : p + 1].to_broadcast((P, QCH)), ADD,
                )

            def do_V(p, tt):
                # one pair's 128 V columns for token tile tt
                ps = opsp.tile([P, QCH], f32, tag="pps", name="pps")
                csl = slice(p * P, (p + 1) * P)
                for di in range(NDT):
                    nc.tensor.matmul(
                        ps[:, 0:P], xt_sb[:, di, tt * P : (tt + 1) * P],
                        wv_ch[:, di, csl],
                        start=(di == 0), stop=(di == NDT - 1),
                    )
                nc.vector.tensor_tensor(
                    vp_sb[tt][:, 2 * p : 2 * p + 2, 0:DK],
                    ps[:, 0:P].rearrange("q (h d) -> q h d", d=DK),
                    bv_rep[:, csl].rearrange("q (h d) -> q h d", d=DK),
                    ADD,
                )

            out_v = out[:].rearrange("(tt p) d -> p tt d", p=P)
            part_tiles = {}

            def do_OP(ch, tg):
                # 2 token tiles -> one grouped result tile -> one DMA
                res = resp.tile([P, 2, QCH], mybir.dt.float16, tag="ores", name="ores")
                for k in range(2):
                    ttk = 2 * tg + k
                    ps = opsp.tile([P, QCH], f32, tag="pps", name="pps")
                    for p in range(NPAIR):
                        nc.tensor.matmul(
                            ps[:], ob_sb[p][:, ttk, :], wo_ch[:, ch, p, :],
                            start=(p == 0), stop=(p == NPAIR - 1),
                        )
                    nc.vector.tensor_tensor(
                        res[:, k, :], ps[:], bo_rep[:, ch * QCH : (ch + 1) * QCH], ADD
                    )
                nc.sync.dma_start(
                    out_v[:, 2 * tg : 2 * tg + 2, ch * QCH : (ch + 1) * QCH], res[:]
                )

            def do_OPP(ch, tg):
                # partial out-proj (pairs 0..2) + bias, staged to SBUF so the
                # final (pair-3) contribution is all that's left for the tail
                part = partp.tile([P, 2, QCH], bf16, tag="part", name="part")
                for k in range(2):
                    ttk = 2 * tg + k
                    ps = opsp.tile([P, QCH], f32, tag="pps", name="pps")
                    for p in range(NPAIR - 1):
                        nc.tensor.matmul(
                            ps[:], ob_sb[p][:, ttk, :], wo_ch[:, ch, p, :],
                            start=(p == 0), stop=(p == NPAIR - 2),
                        )
                    nc.vector.tensor_tensor(
                        part[:, k, :], ps[:], bo_rep[:, ch * QCH : (ch + 1) * QCH], ADD
                    )
                part_tiles[(ch, tg)] = part

            def do_OPF(ch, tg):
                # tail finals: the staged partial joins the accumulation via
                # an identity matmul, and the PSUM->SBUF copies alternate
                # between Act (idle once the exp stream drains) and DVE
                part = part_tiles.pop((ch, tg))
                res = resp.tile([P, 2, QCH], mybir.dt.float16, tag="ores", name="ores")
                for k in range(2):
                    ttk = 2 * tg + k
                    ps = opsp.tile([P, QCH], f32, tag="pps", name="pps")
                    nc.tensor.matmul(
                        ps[:], ob_sb[NPAIR - 1][:, ttk, :], wo_ch[:, ch, NPAIR - 1, :],
                        start=True, stop=False,
                    )
                    nc.tensor.matmul(
                        ps[:], id_sb[:], part[:, k, :], start=False, stop=True,
                    )
                    if k == 0:
                        nc.scalar.activation(res[:, k, :], ps[:], FT.Copy)
                    else:
                        nc.vector.tensor_copy(res[:, k, :], ps[:])
                    # per-half DMAs: the kernel's final transfer is half-size
                    nc.sync.dma_start(
                        out_v[:, ttk, ch * QCH : (ch + 1) * QCH], res[:, k, :]
                    )

            COSTS = {"K": 4096, "Q": 4096, "V": 1024, "OP": 4096, "OPP": 3072,
                     "OPF": 1024}
            EMIT = {"K": do_K, "Q": do_Q, "V": do_V, "OP": do_OP, "OPP": do_OPP,
                    "OPF": do_OPF}

            state = {"budget": 0}
            emitted = set()
            queue = []  # ordered filler keys

            def emit_item(key):
                if key in emitted:
                    return
                emitted.add(key)
                EMIT[key[0]](*key[1:])
                state["budget"] -= COSTS[key[0]]

            def pump(margin=1 << 30):
                # never start an item that would overdraft the period budget
                # by more than `margin`: a 4096-cycle projection on a nearly
                # spent budget stacks ~2 periods of PE work ahead of the next
                # scores and stalls the exp stream
                while queue and state["budget"] > 0:
                    key = queue[0]
                    if key in emitted:
                        queue.pop(0)
                        continue
                    if COSTS[key[0]] > state["budget"] + margin:
                        break
                    queue.pop(0)
                    emit_item(key)

            # filler queue: V per (pair, tt) so early blocks only need pair 0's
            # V; K chunks get pull-emitted exactly when scores need them.
            for p in range(NPAIR):
                for ch in range(4):
                    queue.append(("K", p, ch))
                for tt in range(NKT):
                    queue.append(("V", p, tt))

            # ---------------- phase 2 machinery ----------------
            blocks = [(c, p) for p in range(NPAIR) for c in range(NQC)]
            pt_tiles = {}     # (bi, g, head) -> pt AP
            av_pending = []   # (bi, qt) in emission order
            norm_cnt = [0] * NQC

            def emit_scores(bi, g):
                c, p = blocks[bi]
                qsl = slice(c * QCH, (c + 1) * QCH)
                for head, base, tag in ((0, 0, "sgA"), (1, 64, "sgB")):
                    sg = sgp.tile([P, 2, QCH], f32, tag=tag, name=tag)
                    for j in range(2):
                        kt = 2 * g + j
                        ksl = slice(kt * P, (kt + 1) * P)
                        nc.tensor.matmul(
                            sg[:, j, :],
                            kt_f8[p][base : base + DK, :, ksl],
                            qt_f8[p][base : base + DK, :, qsl],
                            start=True, stop=True,
                            perf_mode=DR,
                            tile_position=(base, 0),
                        )
                    pt = ptp.tile([P, 2, QCH], bf16, tag=f"pt{head}", name="pt")
                    nc.scalar.activation(pt[:], sg[:], FT.Exp, scale=0.125)
                    pt_tiles[(bi, g, head)] = pt
                state["budget"] -= 1024

            def emit_chain(bi, qt):
                # AV for one query tile: per head, a 16-kt accumulation chain
                # in an exclusive PSUM bank (one open group per 2KB zero
                # region), then normalize + transpose out.
                c, p = blocks[bi]
                if qt == 0:
                    for tt in range(NKT):
                        emit_item(("V", p, tt))
                qsl = slice(qt * P, (qt + 1) * P)
                opk = opkp.tile([P, P], bf16, tag="opk", name="opk")
                for head in (0, 1):
                    acc = accp.tile([P, QCH], f32, tag=f"acc{head}", name="acc")
                    h = 2 * p + head
                    for kt in range(NKT):
                        nc.tensor.matmul(
                            acc[:, 0 : DK + 1],
                            pt_tiles[(bi, kt // 2, head)][:, kt % 2, qsl],
                            vp_sb[kt][:, h, :],
                            start=(kt == 0),
                            stop=(kt == NKT - 1),
                        )
                    rec = recp.tile([P, 1], f32, tag=f"rec{head}", name="rec")
                    nc.vector.reciprocal(rec[:], acc[:, DK : DK + 1])
                    nc.vector.tensor_tensor(
                        opk[:, head * DK : (head + 1) * DK], acc[:, 0:DK],
                        rec[:].to_broadcast((P, DK)), MUL,
                    )
                if bi == len(blocks) - 1:
                    # tail: PE transpose + DVE copy (~0.6us) instead of the
                    # ~2.3us xbar-DMA latency chain
                    tp = opsp.tile([P, P], bf16, tag="pps", name="tp")
                    nc.tensor.matmul(
                        tp[:], opk[:], id_sb[:], start=True, stop=True,
                        is_transpose=True,
                    )
                    nc.vector.tensor_copy(ob_sb[p][:, c * NQC + qt, :], tp[:])
                else:
                    nc.sync.dma_start_transpose(ob_sb[p][:, c * NQC + qt, :], opk[:])
                state["budget"] -= 2080
                if c == NQC - 1 and p == NPAIR - 1 and qt in (1, NQC - 1):
                    # last block: queue each final as soon as its own token
                    # tiles are transposed so finals overlap later chains
                    for ch in range(2):
                        queue.append(("OPF", ch, 2 * c + (0 if qt == 1 else 1)))
                if qt == NQC - 1:
                    for g in range(NG):
                        for head in (0, 1):
                            del pt_tiles[(bi, g, head)]
                    norm_cnt[c] += 1
                    if c == NQC - 1:
                        # last chunk: staged partials once pairs 0..2 done
                        if norm_cnt[c] == NPAIR - 1:
                            for ch in range(2):
                                for tg in (2 * c, 2 * c + 1):
                                    queue.append(("OPP", ch, tg))
                    elif norm_cnt[c] == NPAIR:
                        for ch in range(2):
                            for tg in (2 * c, 2 * c + 1):
                                queue.append(("OP", ch, tg))

            def drain_av():
                cap = 1 if len(av_pending) <= NQC else 2
                n = 0
                while av_pending and n < cap:
                    bi, qt = av_pending[0]
                    if exp_done[0] < (bi + 1) * NG:
                        break
                    av_pending.pop(0)
                    emit_chain(bi, qt)
                    n += 1

            exp_done = [0]

            # ---------------- prologue + main loop ----------------
            # PE p-state warmup: the clock ramps only while the engine is
            # continuously busy, so chew on a zeroed scratch tile during the
            # initial DMA wait to enter the first projections at full speed.
            warm = big.tile([P, QCH], bf16, name="warm")
            nc.vector.memset(warm[:], 0.0)
            for w in range(10):
                wps = opsp.tile([P, QCH], f32, tag="pps", name="pps")
                nc.tensor.matmul(
                    wps[:], warm[:, 0:P], warm[:], start=True, stop=True,
                )

            emit_item(("Q", 0, 0))
            emit_item(("K", 0, 0))
            state["budget"] = PROLOGUE_BUDGET

            for bi, (c, p) in enumerate(blocks):
                emit_item(("Q", c, p))
                for g in range(NG):
                    emit_item(("K", p, g // 2))
                    emit_scores(bi, g)
                    # pull the next K chunk right after this period's scores,
                    # two periods ahead of the scores that will need it
                    emit_item(("K", p, min(NG // 2 - 1, g // 2 + 1)))
                    exp_done[0] += 1
                    # pre-pull the next block's projections (spread across two
                    # early periods) so its first scores are never gated on a
                    # just-emitted K/Q and the burst never delays this block's
                    # own next scores by more than a period
                    if bi + 1 < len(blocks):
                        cn, pn = blocks[bi + 1]
                        if g == 0:
                            emit_item(("K", pn, 0))
                        elif g == 2:
                            emit_item(("Q", cn, pn))
                    drain_av()
                    state["budget"] += SLACK_CYC if bi else SLACK_CYC - 1600
                    pump()
                    if state["budget"] > BUDGET_CAP:
                        state["budget"] = BUDGET_CAP
                for qt in range(NQC):
                    av_pending.append((bi, qt))

            # tail: drain chains with the pump interleaved so out-proj
            # finals start as soon as their token tiles are transposed
            while av_pending:
                bi, qt = av_pending.pop(0)
                state["budget"] = 4000
                emit_chain(bi, qt)
                pump(margin=1 << 30)
            state["budget"] = 1 << 30
            pump(margin=1 << 30)

    nc.compile()
    return nc


def _prep_inputs(x, Wq, bq, Wk, bk, Wv, bv, Wo, bo):
    """Shard + lay out inputs for the 8 cores (batch x head-group)."""
    x = np.asarray(x, dtype=np.float32)
    to_bf = lambda a: np.ascontiguousarray(a).astype(ml_dtypes.bfloat16)
    Wq, Wk, Wv, Wo = (np.asarray(w, np.float32) for w in (Wq, Wk, Wv, Wo))
    bq, bv, bo = (np.asarray(v, np.float32) for v in (bq, bv, bo))
    bo_half = np.ascontiguousarray((bo * 0.5).reshape(1, D)).astype(
        ml_dtypes.bfloat16
    )
    xTb = [to_bf(x[b].T) for b in range(B)]
    in_maps = []
    for core in range(NCORES):
        b, hg = core // 2, core % 2
        csl = slice(hg * DC, (hg + 1) * DC)

        def tile_qk(W):
            # [D, DC] -> [p, dt, a, m]
            return to_bf(
                W[:, csl].reshape(NDT, P, NHT, P).transpose(1, 2, 0, 3)
            )

        in_maps.append(
            {
                "xT": xTb[b],
                "Wq": tile_qk(Wq),
                "Wk": tile_qk(Wk),
                "Wv": to_bf(Wv[:, csl].reshape(NDT, P, DC).transpose(1, 0, 2)),
                "Wo": to_bf(
                    Wo[csl, :].reshape(NHT, P, 2, QCH).transpose(1, 2, 0, 3)
                ),
                "bqp": np.ascontiguousarray(bq[csl].reshape(NHT, P).T),
                "bv": np.ascontiguousarray(bv[csl].reshape(1, DC)),
                "ident": np.eye(P, dtype=ml_dtypes.bfloat16),
                "bo": bo_half,
            }
        )
    return in_maps


def kernel(x, Wq, bq, Wk, bk, Wv, bv, Wo, bo):
    if "nc" not in _CACHE:
        _CACHE["nc"] = build_kernel()
    nc = _CACHE["nc"]
    in_maps = _prep_inputs(x, Wq, bq, Wk, bk, Wv, bv, Wo, bo)
    res = run_bass_kernel_spmd(nc, in_maps, list(range(NCORES)))
    out = np.empty((B, T, D), dtype=np.float32)
    for b in range(B):
        out[b] = res.results[2 * b]["out"].astype(np.float32) + res.results[
            2 * b + 1
        ]["out"].astype(np.float32)
    return out
